# revision 4
# baseline (speedup 1.0000x reference)
"""Trainium2 Bass kernel for DiagTrainableLDAHead (retrieval_knn) — v2.

out[n,c] = log_prior[c] - 0.5*(m2[n,c] + log_det)
m2[n,c]  = sum_d (z[n,d]-mu[c,d])^2 * inv_var[d]
=> out[n,c] = cross[n,c] + rb[n] + cb[c]
   cross = (z*inv_var) @ mu.T        fp8e4 DoubleRow GEMM (2x PE rate)
   rb[n] = -0.5 * sum_d z[n,d]^2 inv_var[d]
   cb[c] = log_prior[c] - 0.5*(mu_sq[c] + log_det)

The baseline was HBM-bound (14 MB/core at ~200 GB/s ≈ 71 us).  v2 cuts
traffic to 6 MB/core: z ships as bf16 (1 MB), mu as fp8e4 (1 MB), the
output as fp16 (4 MB; widened to fp32 on host — dtype-only change, the
2e-2 gate has ~7.0 absmax headroom and this path measures 3.0e-3).

Biases are injected into PSUM by a rank-2 f32r matmul (lhsT=[rb;1],
rhs=[1;cb]) opening each accumulation group, so the epilogue is a single
PSUM->fp16 cast pass, rotated across Scalar/Vector/GpSimd.

Sharding: data-parallel over N across 8 cores (1024 rows each);
mu/log_cov_diag/prior_logits replicated.  Host prep is layout/dtype only
(transposes + bf16/fp8 casts); all arithmetic is on-device.
"""
import sys

sys.path.insert(0, "/opt/trn_rl_repo")

import ml_dtypes
import numpy as np

import concourse.bacc as bacc
import concourse.tile as tile
from concourse import mybir
from concourse.bass_utils import run_bass_kernel_spmd

F32 = mybir.dt.float32
F32R = mybir.dt.float32r
BF16 = mybir.dt.bfloat16
F16 = mybir.dt.float16
F8 = mybir.dt.float8e4
AF = mybir.ActivationFunctionType
ALU = mybir.AluOpType
DR = mybir.MatmulPerfMode.DoubleRow

N, C, D = 8192, 2048, 512
NCORES = 8
NSH = N // NCORES          # 1024 rows per core
P = 128                    # partitions
KT = D // P                # 4 k-tiles
KP = KT // 2               # 2 DoubleRow k-pairs
NT = NSH // P              # 8 n-tiles
F = 512                    # c-chunk (PSUM bank width)
CJ = C // F                # 4 c-chunks
ZW = 256                   # z n-chunk width
ZC = NSH // ZW             # 4 z-chunks

_CACHE = {}


def _build():
    nc = bacc.Bacc("TRN2", target_bir_lowering=False, debug=False,
                   enable_asserts=False, num_devices=NCORES)

    zT = nc.dram_tensor("zT", [D, NSH], BF16, kind="ExternalInput").ap()
    muT = nc.dram_tensor("muT", [D, C], F8, kind="ExternalInput").ap()
    lc = nc.dram_tensor("lc", [D], F32, kind="ExternalInput").ap()
    prior = nc.dram_tensor("prior", [C], F32, kind="ExternalInput").ap()
    out = nc.dram_tensor("out", [NSH, C], F16, kind="ExternalOutput").ap()

    with tile.TileContext(nc) as tc:
        with (
            tc.tile_pool(name="const", bufs=1) as const,
            tc.tile_pool(name="sq", bufs=2) as sq,
            tc.tile_pool(name="stage", bufs=2) as stage,
            tc.tile_pool(name="psS", bufs=2, space="PSUM") as psS,
            tc.tile_pool(name="psZ", bufs=2, space="PSUM") as psZ,
            tc.tile_pool(name="psM", bufs=4, space="PSUM") as psM,
        ):
            # ---- small constants (scalar queue) -----------------------
            lc_f = const.tile([1, D], F32)
            nc.scalar.dma_start(out=lc_f[:], in_=lc.rearrange("(a d) -> a d", a=1))
            pr = const.tile([1, C], F32)
            nc.scalar.dma_start(out=pr[:], in_=prior.rearrange("(a c) -> a c", a=1))

            # log_cov in partition layout [p, kt] via PE transposes
            id1 = const.tile([1, 1], F32)
            nc.vector.memset(id1[:], 1.0)
            plc = psZ.tile([P, KT], F32, tag="zchain")
            for kt in range(KT):
                nc.tensor.transpose(plc[:, kt:kt + 1],
                                    lc_f[:, kt * P:(kt + 1) * P], id1[:])
            lc_p = const.tile([P, KT], F32)
            nc.scalar.copy(lc_p[:], plc[:])

            iv_act = const.tile([P, KT], F32)   # exp(-lc), ACT scale operand
            nc.scalar.activation(iv_act[:], lc_p[:], AF.Exp, scale=-1.0)
            iv_bf = const.tile([P, KT], BF16)   # bf16 copy, PE-reduce weights
            nc.scalar.activation(iv_bf[:], lc_p[:], AF.Exp, scale=-1.0)

            # log_det = sum(lc)
            ldsum = const.tile([1, 1], F32)
            nc.vector.tensor_reduce(out=ldsum[:], in_=lc_f[:],
                                    axis=mybir.AxisListType.X, op=ALU.add)
            nldh = const.tile([1, 1], F32)      # -0.5 * log_det
            nc.scalar.mul(nldh[:], ldsum[:], -0.5)

            # log_prior = prior - max - log(sum(exp(prior - max)))
            pmax = const.tile([1, 1], F32)
            nc.vector.tensor_reduce(out=pmax[:], in_=pr[:],
                                    axis=mybir.AxisListType.X, op=ALU.max)
            npmax = const.tile([1, 1], F32)
            nc.scalar.mul(npmax[:], pmax[:], -1.0)
            pexp = const.tile([1, C], F32)
            nc.scalar.activation(pexp[:], pr[:], AF.Exp, bias=npmax[:], scale=1.0)
            psum_e = const.tile([1, 1], F32)
            nc.vector.tensor_reduce(out=psum_e[:], in_=pexp[:],
                                    axis=mybir.AxisListType.X, op=ALU.add)
            lse = const.tile([1, 1], F32)
            nc.scalar.activation(lse[:], psum_e[:], AF.Ln)
            nb = const.tile([1, 1], F32)        # -(lse + pmax)
            nc.scalar.activation(nb[:], lse[:], AF.Identity, bias=pmax[:], scale=1.0)
            nc.scalar.mul(nb[:], nb[:], -1.0)
            lp = const.tile([1, C], F32)        # log_prior
            nc.scalar.activation(lp[:], pr[:], AF.Identity, bias=nb[:], scale=1.0)

            # ---- streamed inputs + bias rows --------------------------
            muT8 = const.tile([P, KT, C], F8)    # mu^T fp8
            zBF = const.tile([P, KT, NSH], BF16)  # z^T bf16
            zT8 = const.tile([P, KT, NSH], F8)   # (z*inv_var)^T fp8
            eRt = const.tile([1, C], F32)
            eR = const.tile([1, C], F32R)
            # bias matmul operands: psum_tile = biasL^T @ biasR
            #   biasL[0,:]=rb, biasL[1,:]=1 ; biasR[0,:]=1, biasR[1,:]=cb
            # engines can only write partition-0-based APs, so row 0 is
            # written in place and row 1 of biasR is fed by a tiny
            # SBUF->SBUF DMA (DMA has no partition-base restriction).
            ones2 = const.tile([2, C], F32)
            nc.vector.memset(ones2[:], 1.0)
            biasL = const.tile([2, NSH], F32R)
            nc.scalar.copy(biasL[:], ones2[:, :NSH])
            biasR = const.tile([2, C], F32R)
            nc.scalar.copy(biasR[:], ones2[:])

            def load_mu(cj):
                s = slice(cj * F, (cj + 1) * F)
                nc.sync.dma_start(out=muT8[:, :, s],
                                  in_=muT[:, s]
                                  .rearrange("(t p) c -> p t c", p=P))
                sqm = sq.tile([P, KT, F], BF16, tag="sqm")
                if cj % 2 == 0:
                    nc.vector.tensor_tensor(sqm[:], muT8[:, :, s],
                                            muT8[:, :, s], ALU.mult)
                else:
                    nc.scalar.activation(sqm[:], muT8[:, :, s], AF.Square)
                pmu = psS.tile([P, F], F32, tag="setup")
                for kt in range(KT):
                    nc.tensor.matmul(pmu[0:1, :], lhsT=iv_bf[:, kt:kt + 1],
                                     rhs=sqm[:, kt, :],
                                     start=(kt == 0), stop=(kt == KT - 1))
                # cb[c] = log_prior[c] - 0.5*(mu_sq[c] + log_det)
                nc.scalar.activation(eRt[:, s], pmu[0:1, :],
                                     AF.Identity, bias=nldh[:], scale=-0.5)
                nc.vector.tensor_tensor(eR[:, s], eRt[:, s], lp[:, s],
                                        ALU.add)
                nc.scalar.dma_start(out=biasR[1:2, s], in_=eR[:, s])

            def load_z(zi):
                s = slice(zi * ZW, (zi + 1) * ZW)
                nc.sync.dma_start(out=zBF[:, :, s],
                                  in_=zT[:, s]
                                  .rearrange("(t p) n -> p t n", p=P))
                # fp8 GEMM operand: (z * inv_var) per k-tile (ACT scale)
                for kt in range(KT):
                    nc.scalar.activation(zT8[:, kt, s], zBF[:, kt, s],
                                         AF.Copy, scale=iv_act[:, kt:kt + 1])
                zq = sq.tile([P, KT, ZW], BF16, tag="zq")
                nc.vector.tensor_tensor(zq[:], zBF[:, :, s], zBF[:, :, s],
                                        ALU.mult)
                pz = psZ.tile([P, ZW], F32, tag="zchain")
                for kt in range(KT):
                    nc.tensor.matmul(pz[0:1, :], lhsT=iv_bf[:, kt:kt + 1],
                                     rhs=zq[:, kt, :],
                                     start=(kt == 0), stop=(kt == KT - 1))
                nc.scalar.activation(biasL[0:1, s], pz[0:1, :], AF.Copy,
                                     scale=-0.5)

            # ---- main GEMM: psum = bias + cross, epilogue = fp16 cast --
            def main_row(ni):
                ot = stage.tile([P, C], F16)
                for cj in range(CJ):
                    s = slice(cj * F, (cj + 1) * F)
                    ps = psM.tile([P, F], F32)
                    nc.tensor.matmul(ps[:],
                                     lhsT=biasL[:, ni * P:(ni + 1) * P],
                                     rhs=biasR[:, s],
                                     start=True, stop=False,
                                     skip_group_check=True)
                    for j in range(KP):
                        nc.tensor.matmul(
                            ps[:],
                            lhsT=zT8[:, 2 * j:2 * j + 2, ni * P:(ni + 1) * P],
                            rhs=muT8[:, 2 * j:2 * j + 2, s],
                            start=False, stop=(j == KP - 1),
                            perf_mode=DR, skip_group_check=True)
                    if cj % 2 == 1:
                        nc.vector.tensor_scalar_add(ot[:, s], ps[:], 0.0)
                    else:
                        nc.scalar.copy(ot[:, s], ps[:])
                nc.gpsimd.dma_start(out=out[ni * P:(ni + 1) * P, :], in_=ot[:])

            # interleave so PE-side setup reduces never sit in front of
            # main matmuls that are already runnable
            load_z(0)
            load_mu(0)
            load_mu(1)
            load_z(1)
            load_mu(2)
            load_mu(3)
            main_row(0)
            main_row(1)
            load_z(2)
            main_row(2)
            main_row(3)
            load_z(3)
            main_row(4)
            main_row(5)
            main_row(6)
            main_row(7)

    nc.compile()
    return nc


def _get_nc():
    if "nc" not in _CACHE:
        _CACHE["nc"] = _build()
    return _CACHE["nc"]


def _in_maps(z, mu, log_cov_diag, prior_logits):
    z = np.ascontiguousarray(np.asarray(z, dtype=np.float32))
    mu = np.asarray(mu, dtype=np.float32)
    lc = np.ascontiguousarray(np.asarray(log_cov_diag, dtype=np.float32))
    pl = np.ascontiguousarray(np.asarray(prior_logits, dtype=np.float32))
    muT = np.ascontiguousarray(mu.T).astype(ml_dtypes.float8_e4m3)
    maps = []
    for c in range(NCORES):
        zTc = np.ascontiguousarray(z[c * NSH:(c + 1) * NSH, :].T) \
            .astype(ml_dtypes.bfloat16)
        maps.append({"zT": zTc, "muT": muT, "lc": lc, "prior": pl})
    return maps


def _run(z, mu, log_cov_diag, prior_logits, trace=False, **kw):
    nc = _get_nc()
    maps = _in_maps(z, mu, log_cov_diag, prior_logits)
    res = run_bass_kernel_spmd(nc, maps, list(range(NCORES)), trace=trace, **kw)
    full = np.concatenate(
        [res.results[c]["out"].astype(np.float32) for c in range(NCORES)],
        axis=0)
    return full, res


def kernel(z, mu, log_cov_diag, prior_logits):
    full, _ = _run(z, mu, log_cov_diag, prior_logits)
    return full


# revision 5
# speedup vs baseline: 1.5945x; 1.5945x over previous
"""Trainium2 Bass kernel for DiagTrainableLDAHead (retrieval_knn) — v3.

out[n,c] = log_prior[c] - 0.5*(m2[n,c] + log_det) = cross[n,c] + rb[n] + cb[c]
   cross = (z*inv_var) @ mu.T        plain fp8e4 GEMM (LDW overlaps MM)
   rb[n] = -0.5 * sum_d z[n,d]^2 inv_var[d]      (per-partition bias, fused
                                                  into the PSUM->fp16 cast)
   cb[c] = log_prior[c] - 0.5*(mu_sq[c]+log_det) (free-dim bias, added in a
                                                  second all-16-bit pass)

Traffic per core 6 MB (vs 14 MB baseline): z bf16 1 MB, mu fp8 1 MB,
out fp16 4 MB (widened to fp32 on host; 2e-2 gate has ~7.0 absmax headroom,
this path measures ~3e-3).

v2 lesson: DoubleRow serializes LDWEIGHTS (no weight double-buffer) and
rank-2 f32r bias matmuls cost 870ns each — both removed.  PE now runs only
plain matmuls: 128 GEMM + 32 reduce + small transposes.

Sharding: data-parallel over N across 8 cores; mu/log_cov/prior replicated.
Host prep is layout/dtype only; all arithmetic on-device.
"""
import sys

sys.path.insert(0, "/opt/trn_rl_repo")

import ml_dtypes
import numpy as np

import concourse.bacc as bacc
import concourse.tile as tile
from concourse import mybir
from concourse.bass_utils import run_bass_kernel_spmd

F32 = mybir.dt.float32
BF16 = mybir.dt.bfloat16
F16 = mybir.dt.float16
F8 = mybir.dt.float8e4
AF = mybir.ActivationFunctionType
ALU = mybir.AluOpType

N, C, D = 8192, 2048, 512
NCORES = 8
NSH = N // NCORES          # 1024 rows per core
P = 128
KT = D // P                # 4 k-tiles
NT = NSH // P              # 8 n-tiles
F = 512                    # c-chunk (PSUM bank width)
CJ = C // F                # 4 c-chunks
ZW = 256                   # z n-chunk width
ZC = NSH // ZW             # 4 z-chunks

_CACHE = {}


def _build():
    nc = bacc.Bacc("TRN2", target_bir_lowering=False, debug=False,
                   enable_asserts=False, num_devices=NCORES)

    zT = nc.dram_tensor("zT", [D, NSH], BF16, kind="ExternalInput").ap()
    muT = nc.dram_tensor("muT", [D, C], F8, kind="ExternalInput").ap()
    lc = nc.dram_tensor("lc", [D], F32, kind="ExternalInput").ap()
    prior = nc.dram_tensor("prior", [C], F32, kind="ExternalInput").ap()
    out = nc.dram_tensor("out", [NSH, C], F16, kind="ExternalOutput").ap()

    with tile.TileContext(nc) as tc:
        with (
            tc.tile_pool(name="const", bufs=1) as const,
            tc.tile_pool(name="sq", bufs=2) as sq,
            tc.tile_pool(name="tmp", bufs=4) as tmp,
            tc.tile_pool(name="stage", bufs=2) as stage,
            tc.tile_pool(name="psS", bufs=2, space="PSUM") as psS,
            tc.tile_pool(name="psZ", bufs=2, space="PSUM") as psZ,
            tc.tile_pool(name="psM", bufs=4, space="PSUM") as psM,
        ):
            # ---- small constants (scalar queue) -----------------------
            lc_f = const.tile([1, D], F32)
            nc.scalar.dma_start(out=lc_f[:], in_=lc.rearrange("(a d) -> a d", a=1))
            pr = const.tile([1, C], F32)
            nc.scalar.dma_start(out=pr[:], in_=prior.rearrange("(a c) -> a c", a=1))

            id1 = const.tile([1, 1], F32)
            nc.vector.memset(id1[:], 1.0)
            plc = psZ.tile([P, KT], F32, tag="zchain")
            for kt in range(KT):
                nc.tensor.transpose(plc[:, kt:kt + 1],
                                    lc_f[:, kt * P:(kt + 1) * P], id1[:])
            lc_p = const.tile([P, KT], F32)
            nc.scalar.copy(lc_p[:], plc[:])

            iv_act = const.tile([P, KT], F32)   # exp(-lc), ACT scale operand
            nc.scalar.activation(iv_act[:], lc_p[:], AF.Exp, scale=-1.0)
            iv_bf = const.tile([P, KT], BF16)   # bf16 copy, PE-reduce weights
            nc.scalar.activation(iv_bf[:], lc_p[:], AF.Exp, scale=-1.0)

            ldsum = const.tile([1, 1], F32)
            nc.vector.tensor_reduce(out=ldsum[:], in_=lc_f[:],
                                    axis=mybir.AxisListType.X, op=ALU.add)
            nldh = const.tile([1, 1], F32)      # -0.5 * log_det
            nc.scalar.mul(nldh[:], ldsum[:], -0.5)

            pmax = const.tile([1, 1], F32)
            nc.vector.tensor_reduce(out=pmax[:], in_=pr[:],
                                    axis=mybir.AxisListType.X, op=ALU.max)
            npmax = const.tile([1, 1], F32)
            nc.scalar.mul(npmax[:], pmax[:], -1.0)
            pexp = const.tile([1, C], F32)
            nc.scalar.activation(pexp[:], pr[:], AF.Exp, bias=npmax[:], scale=1.0)
            psum_e = const.tile([1, 1], F32)
            nc.vector.tensor_reduce(out=psum_e[:], in_=pexp[:],
                                    axis=mybir.AxisListType.X, op=ALU.add)
            lse = const.tile([1, 1], F32)
            nc.scalar.activation(lse[:], psum_e[:], AF.Ln)
            nb = const.tile([1, 1], F32)        # -(lse + pmax)
            nc.scalar.activation(nb[:], lse[:], AF.Identity, bias=pmax[:], scale=1.0)
            nc.scalar.mul(nb[:], nb[:], -1.0)
            lp = const.tile([1, C], F32)        # log_prior
            nc.scalar.activation(lp[:], pr[:], AF.Identity, bias=nb[:], scale=1.0)

            ones_bf = const.tile([1, P], BF16)
            nc.vector.memset(ones_bf[:], 1.0)

            # ---- streamed inputs + bias prep --------------------------
            muT8 = const.tile([P, KT, C], F8)     # mu^T fp8
            zBF = const.tile([P, KT, NSH], BF16)  # z^T bf16
            zT8 = const.tile([P, KT, NSH], F8)    # (z*inv_var)^T fp8
            eRt = const.tile([1, C], F32)
            cb_bf = const.tile([1, C], BF16)      # cb row (rank-1 rhs)
            cb16 = const.tile([P, C], F16)        # cb broadcast to partitions
            rb_p = const.tile([P, NT], F32)       # rb, partition layout
            zsqf = const.tile([1, NSH], F32)      # -0.5*z_sq, free layout

            def load_mu(cj):
                s = slice(cj * F, (cj + 1) * F)
                nc.sync.dma_start(out=muT8[:, :, s],
                                  in_=muT[:, s]
                                  .rearrange("(t p) c -> p t c", p=P))
                sqm = sq.tile([P, KT, F], BF16, tag="sqm")
                if cj % 2 == 0:
                    nc.vector.tensor_tensor(sqm[:], muT8[:, :, s],
                                            muT8[:, :, s], ALU.mult)
                else:
                    nc.scalar.activation(sqm[:], muT8[:, :, s], AF.Square)
                pmu = psS.tile([P, F], F32, tag="setup")
                for kt in range(KT):
                    nc.tensor.matmul(pmu[0:1, :], lhsT=iv_bf[:, kt:kt + 1],
                                     rhs=sqm[:, kt, :],
                                     start=(kt == 0), stop=(kt == KT - 1))
                # cb[c] = log_prior[c] - 0.5*(mu_sq[c] + log_det)
                nc.scalar.activation(eRt[:, s], pmu[0:1, :],
                                     AF.Identity, bias=nldh[:], scale=-0.5)
                nc.vector.tensor_tensor(cb_bf[:, s], eRt[:, s], lp[:, s],
                                        ALU.add)
                # broadcast cb to all partitions (rank-1 matmul), cast fp16
                pcb = psS.tile([P, F], F32, tag="setup")
                nc.tensor.matmul(pcb[:], lhsT=ones_bf[:], rhs=cb_bf[:, s],
                                 start=True, stop=True)
                nc.scalar.copy(cb16[:, s], pcb[:])

            def load_z(zi):
                s = slice(zi * ZW, (zi + 1) * ZW)
                nc.sync.dma_start(out=zBF[:, :, s],
                                  in_=zT[:, s]
                                  .rearrange("(t p) n -> p t n", p=P))
                for kt in range(KT):
                    nc.scalar.activation(zT8[:, kt, s], zBF[:, kt, s],
                                         AF.Copy, scale=iv_act[:, kt:kt + 1])
                zq = sq.tile([P, KT, ZW], BF16, tag="zq")
                nc.vector.tensor_tensor(zq[:], zBF[:, :, s], zBF[:, :, s],
                                        ALU.mult)
                pz = psZ.tile([P, ZW], F32, tag="zchain")
                for kt in range(KT):
                    nc.tensor.matmul(pz[0:1, :], lhsT=iv_bf[:, kt:kt + 1],
                                     rhs=zq[:, kt, :],
                                     start=(kt == 0), stop=(kt == KT - 1))
                nc.scalar.activation(zsqf[:, s], pz[0:1, :], AF.Copy,
                                     scale=-0.5)
                # rb into partition layout via PE transposes
                prb = psZ.tile([P, ZW // P], F32, tag="zchain")
                for li in range(ZW // P):
                    ni = zi * (ZW // P) + li
                    nc.tensor.transpose(prb[:, li:li + 1],
                                        zsqf[:, ni * P:(ni + 1) * P], id1[:])
                nc.scalar.copy(rb_p[:, zi * 2:zi * 2 + 2], prb[:])

            # ---- main GEMM + two-pass epilogue ------------------------
            def main_row(ni):
                ot = stage.tile([P, C], F16)
                for cj in range(CJ):
                    s = slice(cj * F, (cj + 1) * F)
                    ps = psM.tile([P, F], F32)
                    for kt in range(KT):
                        nc.tensor.matmul(
                            ps[:],
                            lhsT=zT8[:, kt, ni * P:(ni + 1) * P],
                            rhs=muT8[:, kt, s],
                            start=(kt == 0), stop=(kt == KT - 1))
                    # pass 1: psum + rb -> fp16 (rb fused as partition bias)
                    t16 = tmp.tile([P, F], F16)
                    if cj % 2 == 0:
                        nc.scalar.activation(t16[:], ps[:], AF.Identity,
                                             bias=rb_p[:, ni:ni + 1], scale=1.0)
                    else:
                        nc.vector.tensor_scalar(t16[:], ps[:],
                                                rb_p[:, ni:ni + 1], None,
                                                ALU.add)
                    # pass 2: + cb (all-16-bit, DVE 2x / gpsimd)
                    if cj < 2:
                        nc.vector.tensor_tensor(ot[:, s], t16[:],
                                                cb16[:, s], ALU.add)
                    else:
                        nc.gpsimd.tensor_tensor(ot[:, s], t16[:],
                                                cb16[:, s], ALU.add)
                nc.gpsimd.dma_start(out=out[ni * P:(ni + 1) * P, :], in_=ot[:])

            load_z(0)
            load_mu(0)
            load_mu(1)
            load_z(1)
            load_mu(2)
            load_mu(3)
            main_row(0)
            main_row(1)
            load_z(2)
            main_row(2)
            main_row(3)
            load_z(3)
            main_row(4)
            main_row(5)
            main_row(6)
            main_row(7)

    nc.compile()
    return nc


def _get_nc():
    if "nc" not in _CACHE:
        _CACHE["nc"] = _build()
    return _CACHE["nc"]


def _in_maps(z, mu, log_cov_diag, prior_logits):
    z = np.ascontiguousarray(np.asarray(z, dtype=np.float32))
    mu = np.asarray(mu, dtype=np.float32)
    lc = np.ascontiguousarray(np.asarray(log_cov_diag, dtype=np.float32))
    pl = np.ascontiguousarray(np.asarray(prior_logits, dtype=np.float32))
    muT = np.ascontiguousarray(mu.T).astype(ml_dtypes.float8_e4m3)
    maps = []
    for c in range(NCORES):
        zTc = np.ascontiguousarray(z[c * NSH:(c + 1) * NSH, :].T) \
            .astype(ml_dtypes.bfloat16)
        maps.append({"zT": zTc, "muT": muT, "lc": lc, "prior": pl})
    return maps


def _run(z, mu, log_cov_diag, prior_logits, trace=False, **kw):
    nc = _get_nc()
    maps = _in_maps(z, mu, log_cov_diag, prior_logits)
    res = run_bass_kernel_spmd(nc, maps, list(range(NCORES)), trace=trace, **kw)
    full = np.concatenate(
        [res.results[c]["out"].astype(np.float32) for c in range(NCORES)],
        axis=0)
    return full, res


def kernel(z, mu, log_cov_diag, prior_logits):
    full, _ = _run(z, mu, log_cov_diag, prior_logits)
    return full
